# revision 1
# baseline (speedup 1.0000x reference)
"""Additive (Bahdanau) attention kernel for Trainium2, 8 NeuronCores.

Problem shapes (hardcoded): B=8, TQ=128, TV=256, D=512, U=256.
Sharding: data-parallel over batch B -> one batch element per core.

Per-core algorithm (all on-chip after the initial DMAs):
  w1vT[u,v]  = (values @ W1)^T           via PE (K=d chunks)
  w2qT[u,q]  = (query  @ W2)^T + (b1+b2) via PE + ACT bias
  for each block of q:
      pre[u,(c,q,v)] = w1vT[u,(c,v)] + w2qT[u,(c,q)]  (DVE broadcast add;
        a couple of q's per block are instead fused into ACT tanh bias)
      feat = tanh(pre) rounded to float32r (ACT, one big-FD instr)
      score pair matmuls (float32r, ~1 cyc/row vs 4 for fp32): V is split
        host-side into tf32-exact Vh+Vl and both accumulate into the same
        PSUM group, so score = (Vh+Vl)@feat = V@feat_r exactly; the only
        precision loss is the single tf32 rounding of tanh (~2.5e-5 rel).
        Each matmul handles a q-pair: V sits at window columns 2p,2p+1 and
        the N=512 rhs is two q's feat; even q's score lands in psum cols
        0:256, odd in 256:512; the unread half of each row is don't-care.
  attn = exp(score) (no max-sub needed; |score| <= sum|V| ~ 13), with an
  even/odd predicated select; rowsums via accum_out; context = attnT^T @
  values scaled by 1/rowsum.  Softmax+context run in two q-halves so the
  first half overlaps the second half's score phase.
  bv is dropped: softmax is shift-invariant.
"""
import sys
import numpy as np

if '/opt/trn_rl_repo' not in sys.path:
    sys.path.insert(0, '/opt/trn_rl_repo')

B, TQ, TV, D, U = 8, 128, 256, 512, 256
P = 128          # partitions
KD = D // P      # 4 k-chunks over d
CU = U // P      # 2 chunks over u
CV = TV // P     # 2 chunks over v
BLOCKS = [16] * 7 + [8] * 2          # q-block sizes (short tail)
assert sum(BLOCKS) == TQ

_compiled = None


def _build():
    import concourse.bass as bass
    import concourse.tile as tile
    from concourse import bacc, mybir

    f32 = mybir.dt.float32
    f32r_ = mybir.dt.float32r
    AF = mybir.ActivationFunctionType

    nc = bacc.Bacc("TRN2", target_bir_lowering=False, debug=False,
                   enable_asserts=True, num_devices=B)

    W1_d = nc.dram_tensor("W1", [P, KD, U], f32, kind="ExternalInput").ap()
    W2_d = nc.dram_tensor("W2", [P, KD, U], f32, kind="ExternalInput").ap()
    QT_d = nc.dram_tensor("QT", [P, KD, TQ], f32, kind="ExternalInput").ap()
    VT_d = nc.dram_tensor("VT", [P, KD, TV], f32, kind="ExternalInput").ap()
    VALH_d = nc.dram_tensor("VALH", [P, CV, D], f32r_, kind="ExternalInput").ap()
    VALL_d = nc.dram_tensor("VALL", [P, CV, D], f32r_, kind="ExternalInput").ap()
    VWH_d = nc.dram_tensor("VWH", [P, CU, 256], f32r_, kind="ExternalInput").ap()
    VWL_d = nc.dram_tensor("VWL", [P, CU, 256], f32r_, kind="ExternalInput").ap()
    B12_d = nc.dram_tensor("B12", [P, CU], f32, kind="ExternalInput").ap()
    ID_d = nc.dram_tensor("ID", [P, P], f32, kind="ExternalInput").ap()
    ME_d = nc.dram_tensor("ME", [P, 1], mybir.dt.uint8,
                          kind="ExternalInput").ap()
    OUT_d = nc.dram_tensor("OUT", [TQ, D], f32, kind="ExternalOutput").ap()

    with tile.TileContext(nc) as tc:
        with (
            tc.tile_pool(name="cst", bufs=1) as cst,
            tc.tile_pool(name="pre_p", bufs=2) as pre_p,
            tc.tile_pool(name="feat_p", bufs=2) as feat_p,
            tc.tile_pool(name="sm", bufs=1) as sm,
            tc.tile_pool(name="ps", bufs=1, space=bass.MemorySpace.PSUM) as ps,
        ):
            f32r = mybir.dt.float32r
            # ---- inputs; chunk the projection operands so matmuls can
            # start before the full tensors arrive ----
            b12 = cst.tile([P, CU], f32, tag="b12")
            nc.gpsimd.dma_start(b12[:], B12_d)
            w1 = cst.tile([P, KD, U], f32, tag="w1")
            vt = cst.tile([P, KD, TV], f32, tag="vt")
            w2 = cst.tile([P, KD, U], f32, tag="w2")
            qt = cst.tile([P, KD, TQ], f32, tag="qt")
            # spread the projection-critical transfers across engine DMA
            # queues so they run in parallel (one queue is ~85 GB/s)
            engs = [nc.sync, nc.scalar, nc.gpsimd, nc.sync]
            for k in range(KD):
                engs[k].dma_start(w1[:, k, :], W1_d[:, k, :])
                engs[k].dma_start(vt[:, k, :], VT_d[:, k, :])
            for k in range(KD):
                engs[k].dma_start(w2[:, k, :], W2_d[:, k, :])
                engs[k].dma_start(qt[:, k, :], QT_d[:, k, :])
            valh = cst.tile([P, CV, D], f32r, tag="valh")
            nc.sync.dma_start(valh[:], VALH_d)
            vall = cst.tile([P, CV, D], f32r, tag="vall")
            nc.sync.dma_start(vall[:], VALL_d)
            vwh = cst.tile([P, CU, 256], f32r, tag="vwh")
            nc.gpsimd.dma_start(vwh[:], VWH_d)
            vwl = cst.tile([P, CU, 256], f32r, tag="vwl")
            nc.gpsimd.dma_start(vwl[:], VWL_d)
            idt = cst.tile([P, P], f32, tag="idt")
            nc.gpsimd.dma_start(idt[:], ID_d)
            mev = cst.tile([P, 1], mybir.dt.uint8, tag="mev")
            nc.gpsimd.dma_start(mev[:], ME_d)


            # ---- projections ----
            # Order: psW2 half 0 first (its ACT copies run while psW1's
            # matmuls continue), then psW1 split by v-half with split copies
            # so block 0's v-split adds can start on the first half.
            # NB: start=True clears has_written for the WHOLE bank, so each
            # group's k-accumulation must complete before the next starts.
            psW1 = ps.tile([P, CU, TV], f32, tag="psW1")   # one bank
            psW2 = ps.tile([P, CU, TQ], f32, tag="psW2")   # half bank
            w1vT = cst.tile([P, CU, TV], f32, tag="w1vT")
            w2qT = cst.tile([P, CU, TQ], f32, tag="w2qT")

            def project_w2(qh):
                qs = slice(qh * 64, qh * 64 + 64)
                for c in range(CU):
                    for k in range(KD):
                        nc.tensor.matmul(psW2[:, c, qs],
                                         w2[:, k, c * P:(c + 1) * P],
                                         qt[:, k, qs],
                                         start=(k == 0), stop=(k == KD - 1))
                for c in range(CU):
                    nc.scalar.activation(w2qT[:, c, qs], psW2[:, c, qs],
                                         AF.Identity, bias=b12[:, c:c + 1])

            def project_w1(vh):
                vs = slice(vh * P, vh * P + P)
                for c in range(CU):
                    for k in range(KD):
                        nc.tensor.matmul(psW1[:, c, vs],
                                         w1[:, k, c * P:(c + 1) * P],
                                         vt[:, k, vs],
                                         start=(k == 0), stop=(k == KD - 1))
                nc.scalar.copy(w1vT[:, :, vs], psW1[:, :, vs])

            project_w2(0)
            project_w1(0)
            project_w1(1)

            # ---- score phase (two psum groups: q<64 and q>=64) ----
            score_A = ps.tile([P, 2 * TV], f32, tag="scoreA")  # one bank
            score_B = ps.tile([P, 2 * TV], f32, tag="scoreB")  # one bank
            att = sm.tile([P, TV], f32, tag="att")
            sums = sm.tile([P, 4], f32, tag="sums")
            psT = ps.tile([P, CV, P], f32, tag="psT")          # half bank
            attnT_h = sm.tile([P, CV, P], f32r, tag="attnT_h")
            attnT_l = sm.tile([P, CV, P], f32r, tag="attnT_l")
            nc.gpsimd.memset(attnT_h[:].bitcast(f32), 0.0)
            nc.gpsimd.memset(attnT_l[:].bitcast(f32), 0.0)
            ctx_ps = ps.tile([P, D], f32, tag="ctx")           # one bank
            ctx = sm.tile([P, D], f32, tag="ctxsb")
            att_e = sm.tile([P, TV], f32, tag="att_e")
            att_o = sm.tile([P, TV], f32, tag="att_o")

            def softmax_context(half):
                """Softmax + transpose + context matmul for one q-half."""
                h0 = half * 64
                score_ps = score_A if half == 0 else score_B
                nc.scalar.activation(att_e[:], score_ps[:, 0:TV], AF.Exp,
                                     accum_out=sums[:, 0:1])
                nc.scalar.activation(att_o[:], score_ps[:, TV:2 * TV], AF.Exp,
                                     accum_out=sums[:, 1:2])
                nc.vector.tensor_copy(att[:], att_o[:])
                nc.vector.copy_predicated(att[:],
                                          mev[:].broadcast_to([P, TV]),
                                          att_e[:])
                nc.vector.tensor_copy(sums[:, 2:3], sums[:, 1:2])
                nc.vector.copy_predicated(sums[:, 2:3], mev[:], sums[:, 0:1])
                nc.vector.reciprocal(sums[:, 3:4], sums[:, 2:3])
                for c in range(CV):
                    nc.tensor.transpose(psT[:, c, h0:h0 + 64],
                                        att[h0:h0 + 64, c * P:(c + 1) * P],
                                        idt[h0:h0 + 64, h0:h0 + 64])
                # exact context via hi/lo f32r: attn = ah + al (al exact in
                # fp32, then tf32-rounded: residual ~2^-26), values = vh + vl
                # host-split; dropped al@vl term is ~2^-26.
                nc.scalar.copy(attnT_h[:, :, h0:h0 + 64],
                               psT[:, :, h0:h0 + 64])
                nc.vector.tensor_sub(attnT_l[:, :, h0:h0 + 64],
                                     psT[:, :, h0:h0 + 64],
                                     attnT_h[:, :, h0:h0 + 64].bitcast(f32))
                # full M=128 lhsT (f32r matmuls cannot target dst partition
                # 64); the other half's columns produce don't-care rows that
                # the next half's start=True clears.
                pairs = [(attnT_h, valh), (attnT_h, vall), (attnT_l, valh)]
                nmm = len(pairs) * CV
                i = 0
                for a_t, v_t in pairs:
                    for c in range(CV):
                        nc.tensor.matmul(ctx_ps[:],
                                         a_t[:, c, :],
                                         v_t[:, c, :],
                                         start=(i == 0), stop=(i == nmm - 1))
                        i += 1
                nc.scalar.mul(ctx[h0:h0 + 64, :], ctx_ps[h0:h0 + 64, :],
                              sums[h0:h0 + 64, 3:4])
                nc.sync.dma_start(OUT_d[h0:h0 + 64, :], ctx[h0:h0 + 64, :])

            mmA = [0, (TQ // 4) * CU * 2]   # counter, total for half A
            mmB = [0, (TQ // 4) * CU * 2]
            q0 = 0
            for bi, bq in enumerate(BLOCKS):
                n_act = 2 if bq >= 16 else (1 if bq >= 8 else 0)
                n_dve = bq - n_act
                pre = pre_p.tile([P, CU, 16, TV], f32, tag="pre")
                feat = feat_p.tile([P, CU, 16, TV], f32r, tag="feat")
                # assists first: they only read psW1/w2qT, so ACT can run
                # them while DVE is still working on this block's adds
                for j in range(n_act):
                    ql = n_dve + j
                    q = q0 + ql
                    for c in range(CU):
                        nc.scalar.activation(feat[:, c, ql, :],
                                             psW1[:, c, :], AF.Tanh,
                                             bias=w2qT[:, c, q:q + 1])
                if bi == 0:
                    # v-split: start adding as soon as half of w1vT is ready
                    for vh in range(2):
                        vs = slice(vh * P, vh * P + P)
                        in0 = (w1vT[:, :, vs].unsqueeze(2)
                               .broadcast_to([P, CU, n_dve, P]))
                        in1 = (w2qT[:, :, q0:q0 + n_dve]
                               .unsqueeze(3).broadcast_to([P, CU, n_dve, P]))
                        nc.vector.tensor_add(pre[:, :, 0:n_dve, vs], in0, in1)
                else:
                    # per-q tensor_scalar adds: fp32 single-src SBUF ops hit
                    # the DVE 2x port mode (263ns/[128,256] measured vs
                    # 543ns/q for the 1x broadcast tensor_tensor), and give
                    # per-pair granularity for the tail tanh
                    for ql in range(n_dve):
                        q = q0 + ql
                        for c in range(CU):
                            nc.vector.tensor_scalar_add(
                                pre[:, c, ql, :], w1vT[:, c, :],
                                w2qT[:, c, q:q + 1])
                if bq >= 16:
                    nc.scalar.activation(feat[:, :, 0:n_dve, :],
                                         pre[:, :, 0:n_dve, :], AF.Tanh)
                else:
                    # small tail blocks: per-pair tanh so the score matmuls
                    # can start while later pairs are still activating
                    for pl in range((n_dve + 1) // 2):
                        sl = slice(2 * pl, min(2 * pl + 2, n_dve))
                        nc.scalar.activation(feat[:, :, sl, :],
                                             pre[:, :, sl, :], AF.Tanh)
                for pl in range(bq // 2):
                    q = q0 + 2 * pl
                    score_ps, mmc = (score_A, mmA) if q < 64 else (score_B, mmB)
                    for c in range(CU):
                        rhs = feat[:, c, 2 * pl:2 * pl + 2, :]
                        for w in (vwh, vwl):
                            nc.tensor.matmul(score_ps[:],
                                             w[:, c, 127 - q:255 - q],
                                             rhs,
                                             start=(mmc[0] == 0),
                                             stop=(mmc[0] == mmc[1] - 1))
                            mmc[0] += 1
                q0 += bq
                if bi == 0:
                    project_w2(1)
                if q0 == 64:
                    softmax_context(0)
            softmax_context(1)

    nc.compile()
    return nc


def _tf32_rne(x):
    b = np.asarray(x, np.float32).view(np.uint32)
    b = (b + 0x7FF + ((b >> 12) & 1)) & np.uint32(0xFFFFF000)
    return b.view(np.float32)


def _prep_shared(W1, b1, W2, b2, V, bv):
    Vf = np.asarray(V, np.float32)[:, 0]
    Vh = _tf32_rne(Vf)
    Vl = _tf32_rne(Vf - Vh)
    Vwh = np.zeros((P, CU, 256), np.float32)
    Vwl = np.zeros((P, CU, 256), np.float32)
    for c in range(CU):
        Vwh[:, c, 127] = Vh[c * P:(c + 1) * P]
        Vwh[:, c, 128] = Vh[c * P:(c + 1) * P]
        Vwl[:, c, 127] = Vl[c * P:(c + 1) * P]
        Vwl[:, c, 128] = Vl[c * P:(c + 1) * P]
    b12 = (b1 + b2).astype(np.float32).reshape(CU, P).T.copy()
    ident = np.eye(P, dtype=np.float32)
    maskE = (1 - (np.arange(P) % 2)).astype(np.uint8).reshape(P, 1)
    W1c = np.ascontiguousarray(
        np.asarray(W1, np.float32).reshape(KD, P, U).transpose(1, 0, 2))
    W2c = np.ascontiguousarray(
        np.asarray(W2, np.float32).reshape(KD, P, U).transpose(1, 0, 2))
    return {
        "W1": W1c,
        "W2": W2c,
        "VWH": Vwh,
        "VWL": Vwl,
        "B12": np.ascontiguousarray(b12),
        "ID": ident,
        "ME": maskE,
    }


def kernel(query, values, W1, b1, W2, b2, V, bv, _trace=False, _tmpdir=None):
    global _compiled
    from concourse.bass_utils import run_bass_kernel_spmd

    query = np.asarray(query, np.float32)
    values = np.asarray(values, np.float32)
    shared = _prep_shared(np.asarray(W1), np.asarray(b1), np.asarray(W2),
                          np.asarray(b2), np.asarray(V), np.asarray(bv))

    if _compiled is None:
        _compiled = _build()
    nc = _compiled

    in_maps = []
    for i in range(B):
        m = dict(shared)
        qT = query[i].T.reshape(KD, P, TQ).transpose(1, 0, 2)
        vT = values[i].T.reshape(KD, P, TV).transpose(1, 0, 2)
        vl = values[i].reshape(CV, P, D).transpose(1, 0, 2)
        vh = _tf32_rne(vl)
        m["QT"] = np.ascontiguousarray(qT)
        m["VT"] = np.ascontiguousarray(vT)
        m["VALH"] = np.ascontiguousarray(vh)
        m["VALL"] = np.ascontiguousarray(_tf32_rne(vl - vh))
        in_maps.append(m)

    kw = {}
    if _trace:
        kw.update(trace=True, tmpdir=_tmpdir)
    res = run_bass_kernel_spmd(nc, in_maps, core_ids=list(range(B)), **kw)
    out = np.stack([res.results[i]["OUT"] for i in range(B)], axis=0)
    if _trace:
        kernel._last_trace = res
    return out

